# revision 1
# baseline (speedup 1.0000x reference)
"""BlendShapes model kernel for 8 Trainium2 NeuronCores.

Computation (reference):
    pose_repr = pose[:, 1:].reshape(B, 23, 9) - eye      # (B, J, 9)
    per-joint MLP 9 -> 18 -> 32 -> 8 (ReLU between)      # coff (B, J, 8)
    basis_full = basis[:, None] * mask[:, :, None, None]  # (V, J, 8, 3)
    res = einsum('bjk,vjkc->bvc', coff, basis_full)       # (B, V, 3)

Mapping:
  - Vertices are sharded across the 8 cores (V=6890 padded to 8*864=6912).
  - Each core computes the full MLP with activations laid out transposed
    ([features, batch]) so the final coefficients coff^T [J*8, B] feed the
    big matmul's stationary operand directly - no on-chip transposes.
  - All matmul operands are fp16 (1 cycle/row on the PE; fp32/f32r run at 4).
    basis values (~1e-4) would be subnormal in fp16, so the mask carries an
    exact 2^13 scale (bf16) applied in the on-chip basis*mask product; the
    PSUM->SBUF evacuation multiplies by 2^-13 (exact) while converting to f32.
  - Joints are processed in chunks of 4 (3 for the tail) with block-diagonal
    weights packed on the host, so each MLP layer chunk is one PE matmul.
  - The output (B x Vc*3 slice per core) is PSUM-accumulated over K = 184
    (split 128 + 56), evacuated via ACT/DVE, and streamed to HBM per b-tile.
"""

import numpy as np

N_VERT, N_JOINT, BPJ, BATCH = 6890, 23, 8, 1024
VPAD = 6912  # 8 * 864
VC = VPAD // 8  # 864 vertices per core
VC3 = VC * 3  # 2592
# Main matmul N tiling: bank-aligned 512-wide tiles (+ a 32 tail), grouped in
# pairs that share one 2-bank PSUM tile so weights load once per K chunk.
NT_BOUNDS = [0, 512, 1024, 1536, 2048, 2560, 2592]
NT_PAIRS = [(0, 1), (2, 3), (4, 5)]
NB = BATCH // 128  # 8 b-tiles

# Unified joint chunking: the same joint groups for all three MLP layers so
# every matmul's rhs is an entire [K, :] tile (base partition 0).
CHUNKS = [(0, 4), (4, 8), (8, 12), (12, 16), (16, 20), (20, 23)]
NCH = len(CHUNKS)

def _offsets(mpj):
    offs, col = [], 0
    for js, je in CHUNKS:
        offs.append(col)
        col += (je - js) * mpj
    return offs, col

W1_OFF, W1_TOT = _offsets(18)  # 414
W2_OFF, W2_TOT = _offsets(32)  # 736
W3_OFF, W3_TOT = _offsets(8)   # 184
W2_OFF = [W1_TOT + o for o in W2_OFF]
W3_OFF = [W1_TOT + W2_TOT + o for o in W3_OFF]
W_COLS = W1_TOT + W2_TOT + W3_TOT  # 1334

# bias_all columns: [0:6] L1 bias, [6:12] L2 bias, [12:18] L3 bias (all per
# chunk), [18:24] eye vectors per chunk (for pose_repr = pose - eye).
BIAS_COLS = 24
BSCALE = 8192.0  # 2**13, exact in bf16
DESCALE = 1.0 / 8192.0  # exact in f32

_CACHED = {}


def _build_nc():
    import concourse.tile as tile
    from concourse import bacc, mybir
    from contextlib import ExitStack

    dt = mybir.dt
    f32, f16, bf16 = dt.float32, dt.float16, dt.bfloat16
    AF = mybir.ActivationFunctionType
    ALU = mybir.AluOpType

    nc = bacc.Bacc(None, target_bir_lowering=False)

    pose_t = nc.dram_tensor("pose_t", [207, BATCH], f16, kind="ExternalInput")
    basis_t = nc.dram_tensor("basis_t", [BPJ, VC3], f32, kind="ExternalInput")
    mask3 = nc.dram_tensor("mask3", [N_JOINT, VC3], bf16, kind="ExternalInput")
    w_all = nc.dram_tensor("w_all", [128, W_COLS], f16, kind="ExternalInput")
    bias_all = nc.dram_tensor("bias_all", [128, BIAS_COLS], f32, kind="ExternalInput")
    res = nc.dram_tensor("res", [BATCH, VC3], f32, kind="ExternalOutput")

    with ExitStack() as ctx:
        tc = ctx.enter_context(tile.TileContext(nc))
        const = ctx.enter_context(tc.tile_pool(name="const", bufs=1))
        work = ctx.enter_context(tc.tile_pool(name="work", bufs=1))
        outp = ctx.enter_context(tc.tile_pool(name="outp", bufs=2))
        pmlp = ctx.enter_context(tc.tile_pool(name="pmlp", bufs=2, space="PSUM"))
        pmain = ctx.enter_context(tc.tile_pool(name="pmain", bufs=3, space="PSUM"))

        # ---- input DMAs, all on the sync queue in critical-path order:
        # bias (gates eye-sub), weights, pose (gates MLP), then basis/mask
        # (needed only once the main matmul starts, after the MLP).
        bias_sb = const.tile([128, BIAS_COLS], f32, tag="bias")
        nc.sync.dma_start(out=bias_sb[:], in_=bias_all[:, :])
        w_sb = const.tile([128, W_COLS], f16, tag="w")
        nc.sync.dma_start(out=w_sb[:], in_=w_all[:, :])

        pose_c = []
        for c, (js, je) in enumerate(CHUNKS):
            K = 9 * (je - js)
            t = work.tile([K, BATCH], f16, tag=f"pose_{c}", name=f"pose_{c}")
            nc.sync.dma_start(out=t[:], in_=pose_t[9 * js : 9 * js + K, :])
            pose_c.append(t)

        bf_a = work.tile([128, VC3], f32, tag="bf_a")
        bf_b = work.tile([56, VC3], f32, tag="bf_b")
        mk_a = work.tile([128, VC3], bf16, tag="mk_a")
        mk_b = work.tile([56, VC3], bf16, tag="mk_b")
        bfm_a = work.tile([128, VC3], f16, tag="bfm_a")
        bfm_b = work.tile([56, VC3], f16, tag="bfm_b")
        nc.sync.dma_start(out=bf_a[:], in_=basis_t[:, :].partition_broadcast(16))
        nc.sync.dma_start(
            out=mk_a[:], in_=mask3[0:16, :][:, None, :].broadcast_to([16, BPJ, VC3])
        )
        nc.sync.dma_start(out=bf_b[:], in_=basis_t[:, :].partition_broadcast(7))
        nc.sync.dma_start(
            out=mk_b[:], in_=mask3[16:23, :][:, None, :].broadcast_to([7, BPJ, VC3])
        )

        # pose_repr = pose - eye (in place, fp16, DVE 2x mode)
        for c, (js, je) in enumerate(CHUNKS):
            K = 9 * (je - js)
            nc.vector.tensor_scalar(
                out=pose_c[c][:],
                in0=pose_c[c][:],
                scalar1=bias_sb[0:K, 18 + c : 19 + c],
                scalar2=None,
                op0=ALU.subtract,
            )

        # basis_full = (basis * 2^13) * mask on GPSIMD -> fp16 product tiles
        for t in range(len(NT_BOUNDS) - 1):
            sl = slice(NT_BOUNDS[t], NT_BOUNDS[t + 1])
            nc.gpsimd.tensor_tensor(
                out=bfm_a[:, sl], in0=bf_a[:, sl], in1=mk_a[:, sl], op=ALU.mult
            )
            nc.gpsimd.tensor_tensor(
                out=bfm_b[:, sl], in0=bf_b[:, sl], in1=mk_b[:, sl], op=ALU.mult
            )

        coffT_a = work.tile([128, BATCH], f16, tag="coffT_a")
        coffT_b = work.tile([56, BATCH], f16, tag="coffT_b")
        h1 = {}
        h2 = {}
        coff_c = {}

        def mlp_epilogue(use_act, dst, ps, bias_ap, relu):
            # Split between ACT and DVE so the 2-slot PSUM chain advances two
            # tiles per epilogue latency instead of one. ACT's Copy cannot
            # take an AP bias, so bias-only (L3) epilogues go to DVE.
            if use_act:
                nc.scalar.activation(dst, ps, AF.Relu, bias=bias_ap)
            elif relu:
                nc.vector.tensor_scalar(
                    out=dst, in0=ps, scalar1=bias_ap, scalar2=0.0,
                    op0=ALU.add, op1=ALU.max,
                )
            else:
                nc.vector.tensor_scalar(
                    out=dst, in0=ps, scalar1=bias_ap, scalar2=None, op0=ALU.add
                )

        def mlp_half(h):
            hs = slice(h * 512, (h + 1) * 512)
            # L1: 9nj -> 18nj, ReLU(x + b)
            for c, (js, je) in enumerate(CHUNKS):
                nj = je - js
                K, M = 9 * nj, 18 * nj
                off = W1_OFF[c]
                ps = pmlp.tile([M, 512], f32, tag="psmlp", name=f"ps1_{c}_{h}")
                nc.tensor.matmul(
                    ps[:], lhsT=w_sb[0:K, off : off + M], rhs=pose_c[c][:, hs],
                    start=True, stop=True,
                )
                if h == 0:
                    h1[c] = work.tile([M, BATCH], f16, tag=f"h1_{c}", name=f"h1_{c}")
                mlp_epilogue(True, h1[c][:, hs], ps[:], bias_sb[0:M, c : c + 1], True)
            # L2: 18nj -> 32nj, ReLU
            for c, (js, je) in enumerate(CHUNKS):
                nj = je - js
                K, M = 18 * nj, 32 * nj
                off = W2_OFF[c]
                ps = pmlp.tile([M, 512], f32, tag="psmlp", name=f"ps2_{c}_{h}")
                nc.tensor.matmul(
                    ps[:], lhsT=w_sb[0:K, off : off + M], rhs=h1[c][:, hs],
                    start=True, stop=True,
                )
                if h == 0:
                    h2[c] = work.tile([M, BATCH], f16, tag=f"h2_{c}", name=f"h2_{c}")
                mlp_epilogue(c % 2 == 0, h2[c][:, hs], ps[:], bias_sb[0:M, 6 + c : 7 + c], True)
            # L3: 32nj -> 8nj, bias only, into per-chunk coff tiles; small
            # SBUF->SBUF DMAs (gpsimd queue) merge them into coffT_a/coffT_b
            # (DMA is the only engine that can shift partition bases).
            for c, (js, je) in enumerate(CHUNKS):
                nj = je - js
                K, M = 32 * nj, 8 * nj
                off = W3_OFF[c]
                ps = pmlp.tile([M, 512], f32, tag="psmlp", name=f"ps3_{c}_{h}")
                nc.tensor.matmul(
                    ps[:], lhsT=w_sb[0:K, off : off + M], rhs=h2[c][:, hs],
                    start=True, stop=True,
                )
                if h == 0:
                    coff_c[c] = work.tile(
                        [M, BATCH], f16, tag=f"coff_{c}", name=f"coff_{c}"
                    )
                mlp_epilogue(
                    False, coff_c[c][:, hs], ps[:], bias_sb[0:M, 12 + c : 13 + c], False
                )
                if c < 4:
                    dst = coffT_a[32 * c : 32 * c + M, hs]
                else:
                    r0 = 32 * (c - 4)
                    dst = coffT_b[r0 : r0 + M, hs]
                nc.gpsimd.dma_start(out=dst, in_=coff_c[c][:, hs])

        def main_btile(bt):
            bsl = slice(bt * 128, (bt + 1) * 128)
            ostrip = outp.tile([128, VC3], f32, tag="ostrip", name=f"ostrip_{bt}")
            for p, (t0, t1) in enumerate(NT_PAIRS):
                s0 = slice(NT_BOUNDS[t0], NT_BOUNDS[t0 + 1])
                s1 = slice(NT_BOUNDS[t1], NT_BOUNDS[t1 + 1])
                n0 = NT_BOUNDS[t0 + 1] - NT_BOUNDS[t0]
                n1 = NT_BOUNDS[t1 + 1] - NT_BOUNDS[t1]
                ps = pmain.tile([128, 1024], f32, tag="ps", name=f"ps_{bt}_{p}")
                # K chunk a for both tiles of the pair (weights loaded once),
                # then K chunk b accumulating on top. Tile 1 starts at column
                # 512 so each matmul output stays within one PSUM bank.
                nc.tensor.matmul(
                    ps[:, 0:n0], lhsT=coffT_a[:, bsl], rhs=bfm_a[:, s0],
                    start=True, stop=False,
                )
                nc.tensor.matmul(
                    ps[:, 512 : 512 + n1], lhsT=coffT_a[:, bsl], rhs=bfm_a[:, s1],
                    start=True, stop=False,
                )
                nc.tensor.matmul(
                    ps[:, 0:n0], lhsT=coffT_b[:, bsl], rhs=bfm_b[:, s0],
                    start=False, stop=True,
                )
                nc.tensor.matmul(
                    ps[:, 512 : 512 + n1], lhsT=coffT_b[:, bsl], rhs=bfm_b[:, s1],
                    start=False, stop=True,
                )
                # evacuate PSUM -> SBUF with the exact 2^-13 descale; the two
                # tiles are contiguous in PSUM (columns 0..512+n1) only when
                # n0 == 512, which holds for every pair by construction.
                osl = slice(NT_BOUNDS[t0], NT_BOUNDS[t0] + 512 + n1)
                if (bt * len(NT_PAIRS) + p) % 2 == 0:
                    nc.scalar.activation(
                        ostrip[:, osl], ps[:, 0 : 512 + n1], AF.Copy, scale=DESCALE
                    )
                else:
                    nc.vector.tensor_scalar(
                        out=ostrip[:, osl], in0=ps[:, 0 : 512 + n1], scalar1=DESCALE,
                        scalar2=None, op0=ALU.mult,
                    )
            nc.sync.dma_start(out=res[bsl, :], in_=ostrip[:])

        # First batch-half of the MLP, then its 4 output b-tiles (overlapping
        # the second half's MLP epilogues), then the rest.
        mlp_half(0)
        for bt in range(4):
            main_btile(bt)
        mlp_half(1)
        for bt in range(4, NB):
            main_btile(bt)

    nc.finalize()
    return nc


def _pack_host(pose, basis, mask, w1, b1, w2, b2, w3, b3):
    import ml_dtypes

    pose_t = np.ascontiguousarray(
        pose[:, 1:].reshape(BATCH, 207).T.astype(np.float16)
    )  # [207, B] rows are (j, i)

    basis_t = np.zeros((BPJ, VPAD * 3), np.float32)  # [k, (v, c)]
    basis_t[:, : N_VERT * 3] = basis.transpose(1, 0, 2).reshape(BPJ, N_VERT * 3)

    mask3 = np.zeros((N_JOINT, VPAD * 3), ml_dtypes.bfloat16)  # [j, (v, c)]
    mask3[:, : N_VERT * 3] = (np.repeat(mask.T, 3, axis=1) * BSCALE).astype(
        ml_dtypes.bfloat16
    )

    w_all = np.zeros((128, W_COLS), np.float16)
    bias_all = np.zeros((128, BIAS_COLS), np.float32)
    for (js, je), o1, o2, o3 in zip(CHUNKS, W1_OFF, W2_OFF, W3_OFF):
        for t, j in enumerate(range(js, je)):
            w_all[t * 9 : (t + 1) * 9, o1 + t * 18 : o1 + (t + 1) * 18] = w1[j]
            w_all[t * 18 : (t + 1) * 18, o2 + t * 32 : o2 + (t + 1) * 32] = w2[j]
            w_all[t * 32 : (t + 1) * 32, o3 + t * 8 : o3 + (t + 1) * 8] = w3[j]
    for c, (js, je) in enumerate(CHUNKS):
        nj = je - js
        bias_all[0 : 18 * nj, c] = b1[js:je].reshape(-1)
        bias_all[0 : 32 * nj, 6 + c] = b2[js:je].reshape(-1)
        bias_all[0 : 8 * nj, 12 + c] = b3[js:je].reshape(-1)
        # eye vector for this chunk's pose rows: 1.0 at i in {0, 4, 8}
        ev = np.zeros((nj, 9), np.float32)
        ev[:, [0, 4, 8]] = 1.0
        bias_all[0 : 9 * nj, 18 + c] = ev.reshape(-1)

    return pose_t, basis_t, mask3, w_all, bias_all


def _in_maps(pose, basis, mask, w1, b1, w2, b2, w3, b3):
    pose_t, basis_t, mask3, w_all, bias_all = _pack_host(
        np.asarray(pose, np.float32),
        np.asarray(basis, np.float32),
        np.asarray(mask, np.float32),
        np.asarray(w1, np.float32),
        np.asarray(b1, np.float32),
        np.asarray(w2, np.float32),
        np.asarray(b2, np.float32),
        np.asarray(w3, np.float32),
        np.asarray(b3, np.float32),
    )
    maps = []
    for i in range(8):
        c0 = i * VC3
        maps.append(
            {
                "pose_t": pose_t,
                "basis_t": np.ascontiguousarray(basis_t[:, c0 : c0 + VC3]),
                "mask3": np.ascontiguousarray(mask3[:, c0 : c0 + VC3]),
                "w_all": w_all,
                "bias_all": bias_all,
            }
        )
    return maps


def kernel(pose, basis, mask, w1, b1, w2, b2, w3, b3):
    from concourse.bass_utils import run_bass_kernel_spmd

    if "nc" not in _CACHED:
        _CACHED["nc"] = _build_nc()
    nc = _CACHED["nc"]

    maps = _in_maps(pose, basis, mask, w1, b1, w2, b2, w3, b3)
    r = run_bass_kernel_spmd(nc, maps, core_ids=list(range(8)))
    out = np.concatenate(
        [m["res"].reshape(BATCH, VC, 3) for m in r.results], axis=1
    )
    return np.ascontiguousarray(out[:, :N_VERT, :])



# revision 2
# speedup vs baseline: 1.0009x; 1.0009x over previous
"""BlendShapes model kernel for 8 Trainium2 NeuronCores.

Computation (reference):
    pose_repr = pose[:, 1:].reshape(B, 23, 9) - eye      # (B, J, 9)
    per-joint MLP 9 -> 18 -> 32 -> 8 (ReLU between)      # coff (B, J, 8)
    basis_full = basis[:, None] * mask[:, :, None, None]  # (V, J, 8, 3)
    res = einsum('bjk,vjkc->bvc', coff, basis_full)       # (B, V, 3)

Mapping:
  - Vertices are sharded across the 8 cores (V=6890 padded to 8*864=6912).
  - Each core computes the full MLP with activations laid out transposed
    ([features, batch]) so the final coefficients coff^T [J*8, B] feed the
    big matmul's stationary operand directly - no on-chip transposes.
  - All matmul operands are fp16 (1 cycle/row on the PE; fp32/f32r run at 4).
    basis values (~1e-4) would be subnormal in fp16, so the mask carries an
    exact 2^13 scale (bf16) applied in the on-chip basis*mask product; the
    PSUM->SBUF evacuation multiplies by 2^-13 (exact) while converting to f32.
  - Joints are processed in chunks of 4 (3 for the tail) with block-diagonal
    weights packed on the host, so each MLP layer chunk is one PE matmul.
  - The output (B x Vc*3 slice per core) is PSUM-accumulated over K = 184
    (split 128 + 56), evacuated via ACT/DVE, and streamed to HBM per b-tile.
"""

import numpy as np

N_VERT, N_JOINT, BPJ, BATCH = 6890, 23, 8, 1024
VPAD = 6912  # 8 * 864
VC = VPAD // 8  # 864 vertices per core
VC3 = VC * 3  # 2592
# Main matmul N tiling: bank-aligned 512-wide tiles (+ a 32 tail), grouped in
# pairs that share one 2-bank PSUM tile so weights load once per K chunk.
NT_BOUNDS = [0, 512, 1024, 1536, 2048, 2560, 2592]
NT_PAIRS = [(0, 1), (2, 3), (4, 5)]
NB = BATCH // 128  # 8 b-tiles

# Unified joint chunking: the same joint groups for all three MLP layers so
# every matmul's rhs is an entire [K, :] tile (base partition 0).
CHUNKS = [(0, 4), (4, 8), (8, 12), (12, 16), (16, 20), (20, 23)]
NCH = len(CHUNKS)

def _offsets(mpj):
    offs, col = [], 0
    for js, je in CHUNKS:
        offs.append(col)
        col += (je - js) * mpj
    return offs, col

W1_OFF, W1_TOT = _offsets(18)  # 414
W2_OFF, W2_TOT = _offsets(32)  # 736
W3_OFF, W3_TOT = _offsets(8)   # 184
W2_OFF = [W1_TOT + o for o in W2_OFF]
W3_OFF = [W1_TOT + W2_TOT + o for o in W3_OFF]
W_COLS = W1_TOT + W2_TOT + W3_TOT  # 1334

# bias_all columns: [0:6] L1 bias, [6:12] L2 bias, [12:18] L3 bias (all per
# chunk), [18:24] eye vectors per chunk (for pose_repr = pose - eye).
BIAS_COLS = 24
BSCALE = 8192.0  # 2**13, exact in bf16
DESCALE = 1.0 / 8192.0  # exact in f32

_CACHED = {}


def _build_nc():
    import concourse.tile as tile
    from concourse import bacc, mybir
    from contextlib import ExitStack

    dt = mybir.dt
    f32, f16, bf16 = dt.float32, dt.float16, dt.bfloat16
    AF = mybir.ActivationFunctionType
    ALU = mybir.AluOpType

    nc = bacc.Bacc(None, target_bir_lowering=False)

    pose_t = nc.dram_tensor("pose_t", [207, BATCH], bf16, kind="ExternalInput")
    basis_t = nc.dram_tensor("basis_t", [BPJ, VC3], f32, kind="ExternalInput")
    mask3 = nc.dram_tensor("mask3", [N_JOINT, VC3], bf16, kind="ExternalInput")
    w_all = nc.dram_tensor("w_all", [128, W_COLS], bf16, kind="ExternalInput")
    bias_all = nc.dram_tensor("bias_all", [128, BIAS_COLS], f32, kind="ExternalInput")
    res = nc.dram_tensor("res", [BATCH, VC3], f32, kind="ExternalOutput")

    with ExitStack() as ctx:
        tc = ctx.enter_context(tile.TileContext(nc))
        const = ctx.enter_context(tc.tile_pool(name="const", bufs=1))
        work = ctx.enter_context(tc.tile_pool(name="work", bufs=1))
        outp = ctx.enter_context(tc.tile_pool(name="outp", bufs=2))
        pmlp = ctx.enter_context(tc.tile_pool(name="pmlp", bufs=2, space="PSUM"))
        pmain = ctx.enter_context(tc.tile_pool(name="pmain", bufs=3, space="PSUM"))

        # ---- input DMAs, all on the sync queue in critical-path order:
        # bias (gates eye-sub), weights, pose (gates MLP), then basis/mask
        # (needed only once the main matmul starts, after the MLP).
        bias_sb = const.tile([128, BIAS_COLS], f32, tag="bias")
        nc.sync.dma_start(out=bias_sb[:], in_=bias_all[:, :])
        w_sb = const.tile([128, W_COLS], bf16, tag="w")
        nc.sync.dma_start(out=w_sb[:], in_=w_all[:, :])

        pose_c = []
        for c, (js, je) in enumerate(CHUNKS):
            K = 9 * (je - js)
            t = work.tile([K, BATCH], bf16, tag=f"pose_{c}", name=f"pose_{c}")
            nc.sync.dma_start(out=t[:], in_=pose_t[9 * js : 9 * js + K, :])
            pose_c.append(t)

        bf_a = work.tile([128, VC3], f32, tag="bf_a")
        bf_b = work.tile([56, VC3], f32, tag="bf_b")
        mk_a = work.tile([128, VC3], bf16, tag="mk_a")
        mk_b = work.tile([56, VC3], bf16, tag="mk_b")
        bfm_a = work.tile([128, VC3], bf16, tag="bfm_a")
        bfm_b = work.tile([56, VC3], bf16, tag="bfm_b")
        nc.sync.dma_start(out=bf_a[:], in_=basis_t[:, :].partition_broadcast(16))
        nc.sync.dma_start(
            out=mk_a[:], in_=mask3[0:16, :][:, None, :].broadcast_to([16, BPJ, VC3])
        )
        nc.sync.dma_start(out=bf_b[:], in_=basis_t[:, :].partition_broadcast(7))
        nc.sync.dma_start(
            out=mk_b[:], in_=mask3[16:23, :][:, None, :].broadcast_to([7, BPJ, VC3])
        )

        # pose_repr = pose - eye (in place, fp16, DVE 2x mode)
        for c, (js, je) in enumerate(CHUNKS):
            K = 9 * (je - js)
            nc.vector.tensor_scalar(
                out=pose_c[c][:],
                in0=pose_c[c][:],
                scalar1=bias_sb[0:K, 18 + c : 19 + c],
                scalar2=None,
                op0=ALU.subtract,
            )

        # basis_full = (basis * 2^13) * mask on GPSIMD -> fp16 product tiles
        for t in range(len(NT_BOUNDS) - 1):
            sl = slice(NT_BOUNDS[t], NT_BOUNDS[t + 1])
            nc.gpsimd.tensor_tensor(
                out=bfm_a[:, sl], in0=bf_a[:, sl], in1=mk_a[:, sl], op=ALU.mult
            )
            nc.gpsimd.tensor_tensor(
                out=bfm_b[:, sl], in0=bf_b[:, sl], in1=mk_b[:, sl], op=ALU.mult
            )

        coffT_a = work.tile([128, BATCH], bf16, tag="coffT_a")
        coffT_b = work.tile([56, BATCH], bf16, tag="coffT_b")
        h1 = {}
        h2 = {}
        coff_c = {}

        def mlp_epilogue(use_act, dst, ps, bias_ap, relu):
            # Split between ACT and DVE so the 2-slot PSUM chain advances two
            # tiles per epilogue latency instead of one. ACT's Copy cannot
            # take an AP bias, so bias-only (L3) epilogues go to DVE.
            if use_act:
                nc.scalar.activation(dst, ps, AF.Relu, bias=bias_ap)
            elif relu:
                nc.vector.tensor_scalar(
                    out=dst, in0=ps, scalar1=bias_ap, scalar2=0.0,
                    op0=ALU.add, op1=ALU.max,
                )
            else:
                nc.vector.tensor_scalar(
                    out=dst, in0=ps, scalar1=bias_ap, scalar2=None, op0=ALU.add
                )

        def mlp_half(h):
            hs = slice(h * 512, (h + 1) * 512)
            # L1: 9nj -> 18nj, ReLU(x + b)
            for c, (js, je) in enumerate(CHUNKS):
                nj = je - js
                K, M = 9 * nj, 18 * nj
                off = W1_OFF[c]
                ps = pmlp.tile([M, 512], f32, tag="psmlp", name=f"ps1_{c}_{h}")
                nc.tensor.matmul(
                    ps[:], lhsT=w_sb[0:K, off : off + M], rhs=pose_c[c][:, hs],
                    start=True, stop=True,
                )
                if h == 0:
                    h1[c] = work.tile([M, BATCH], bf16, tag=f"h1_{c}", name=f"h1_{c}")
                mlp_epilogue(True, h1[c][:, hs], ps[:], bias_sb[0:M, c : c + 1], True)
            # L2: 18nj -> 32nj, ReLU
            for c, (js, je) in enumerate(CHUNKS):
                nj = je - js
                K, M = 18 * nj, 32 * nj
                off = W2_OFF[c]
                ps = pmlp.tile([M, 512], f32, tag="psmlp", name=f"ps2_{c}_{h}")
                nc.tensor.matmul(
                    ps[:], lhsT=w_sb[0:K, off : off + M], rhs=h1[c][:, hs],
                    start=True, stop=True,
                )
                if h == 0:
                    h2[c] = work.tile([M, BATCH], bf16, tag=f"h2_{c}", name=f"h2_{c}")
                mlp_epilogue(c % 2 == 0, h2[c][:, hs], ps[:], bias_sb[0:M, 6 + c : 7 + c], True)
            # L3: 32nj -> 8nj, bias only, into per-chunk coff tiles; small
            # SBUF->SBUF DMAs (gpsimd queue) merge them into coffT_a/coffT_b
            # (DMA is the only engine that can shift partition bases).
            for c, (js, je) in enumerate(CHUNKS):
                nj = je - js
                K, M = 32 * nj, 8 * nj
                off = W3_OFF[c]
                ps = pmlp.tile([M, 512], f32, tag="psmlp", name=f"ps3_{c}_{h}")
                nc.tensor.matmul(
                    ps[:], lhsT=w_sb[0:K, off : off + M], rhs=h2[c][:, hs],
                    start=True, stop=True,
                )
                if h == 0:
                    coff_c[c] = work.tile(
                        [M, BATCH], bf16, tag=f"coff_{c}", name=f"coff_{c}"
                    )
                mlp_epilogue(
                    False, coff_c[c][:, hs], ps[:], bias_sb[0:M, 12 + c : 13 + c], False
                )
                if c < 4:
                    dst = coffT_a[32 * c : 32 * c + M, hs]
                else:
                    r0 = 32 * (c - 4)
                    dst = coffT_b[r0 : r0 + M, hs]
                nc.gpsimd.dma_start(out=dst, in_=coff_c[c][:, hs])

        def main_btile(bt):
            bsl = slice(bt * 128, (bt + 1) * 128)
            ostrip = outp.tile([128, VC3], f32, tag="ostrip", name=f"ostrip_{bt}")
            for p, (t0, t1) in enumerate(NT_PAIRS):
                s0 = slice(NT_BOUNDS[t0], NT_BOUNDS[t0 + 1])
                s1 = slice(NT_BOUNDS[t1], NT_BOUNDS[t1 + 1])
                n0 = NT_BOUNDS[t0 + 1] - NT_BOUNDS[t0]
                n1 = NT_BOUNDS[t1 + 1] - NT_BOUNDS[t1]
                ps = pmain.tile([128, 1024], f32, tag="ps", name=f"ps_{bt}_{p}")
                # K chunk a for both tiles of the pair (weights loaded once),
                # then K chunk b accumulating on top. Tile 1 starts at column
                # 512 so each matmul output stays within one PSUM bank.
                nc.tensor.matmul(
                    ps[:, 0:n0], lhsT=coffT_a[:, bsl], rhs=bfm_a[:, s0],
                    start=True, stop=False,
                )
                nc.tensor.matmul(
                    ps[:, 512 : 512 + n1], lhsT=coffT_a[:, bsl], rhs=bfm_a[:, s1],
                    start=True, stop=False,
                )
                nc.tensor.matmul(
                    ps[:, 0:n0], lhsT=coffT_b[:, bsl], rhs=bfm_b[:, s0],
                    start=False, stop=True,
                )
                nc.tensor.matmul(
                    ps[:, 512 : 512 + n1], lhsT=coffT_b[:, bsl], rhs=bfm_b[:, s1],
                    start=False, stop=True,
                )
                # evacuate PSUM -> SBUF with the exact 2^-13 descale; the two
                # tiles are contiguous in PSUM (columns 0..512+n1) only when
                # n0 == 512, which holds for every pair by construction.
                osl = slice(NT_BOUNDS[t0], NT_BOUNDS[t0] + 512 + n1)
                if (bt * len(NT_PAIRS) + p) % 2 == 0:
                    nc.scalar.activation(
                        ostrip[:, osl], ps[:, 0 : 512 + n1], AF.Copy, scale=DESCALE
                    )
                else:
                    nc.vector.tensor_scalar(
                        out=ostrip[:, osl], in0=ps[:, 0 : 512 + n1], scalar1=DESCALE,
                        scalar2=None, op0=ALU.mult,
                    )
            nc.sync.dma_start(out=res[bsl, :], in_=ostrip[:])

        # First batch-half of the MLP, then its 4 output b-tiles (overlapping
        # the second half's MLP epilogues), then the rest.
        mlp_half(0)
        for bt in range(4):
            main_btile(bt)
        mlp_half(1)
        for bt in range(4, NB):
            main_btile(bt)

    nc.finalize()
    return nc


def _pack_host(pose, basis, mask, w1, b1, w2, b2, w3, b3):
    import ml_dtypes

    pose_t = np.ascontiguousarray(
        pose[:, 1:].reshape(BATCH, 207).T.astype(ml_dtypes.bfloat16)
    )  # [207, B] rows are (j, i)

    basis_t = np.zeros((BPJ, VPAD * 3), np.float32)  # [k, (v, c)]
    basis_t[:, : N_VERT * 3] = basis.transpose(1, 0, 2).reshape(BPJ, N_VERT * 3)

    mask3 = np.zeros((N_JOINT, VPAD * 3), ml_dtypes.bfloat16)  # [j, (v, c)]
    mask3[:, : N_VERT * 3] = (np.repeat(mask.T, 3, axis=1) * BSCALE).astype(
        ml_dtypes.bfloat16
    )

    w_all = np.zeros((128, W_COLS), ml_dtypes.bfloat16)
    bias_all = np.zeros((128, BIAS_COLS), np.float32)
    for (js, je), o1, o2, o3 in zip(CHUNKS, W1_OFF, W2_OFF, W3_OFF):
        for t, j in enumerate(range(js, je)):
            w_all[t * 9 : (t + 1) * 9, o1 + t * 18 : o1 + (t + 1) * 18] = w1[j]
            w_all[t * 18 : (t + 1) * 18, o2 + t * 32 : o2 + (t + 1) * 32] = w2[j]
            w_all[t * 32 : (t + 1) * 32, o3 + t * 8 : o3 + (t + 1) * 8] = w3[j]
    for c, (js, je) in enumerate(CHUNKS):
        nj = je - js
        bias_all[0 : 18 * nj, c] = b1[js:je].reshape(-1)
        bias_all[0 : 32 * nj, 6 + c] = b2[js:je].reshape(-1)
        bias_all[0 : 8 * nj, 12 + c] = b3[js:je].reshape(-1)
        # eye vector for this chunk's pose rows: 1.0 at i in {0, 4, 8}
        ev = np.zeros((nj, 9), np.float32)
        ev[:, [0, 4, 8]] = 1.0
        bias_all[0 : 9 * nj, 18 + c] = ev.reshape(-1)

    return pose_t, basis_t, mask3, w_all, bias_all


def _in_maps(pose, basis, mask, w1, b1, w2, b2, w3, b3):
    pose_t, basis_t, mask3, w_all, bias_all = _pack_host(
        np.asarray(pose, np.float32),
        np.asarray(basis, np.float32),
        np.asarray(mask, np.float32),
        np.asarray(w1, np.float32),
        np.asarray(b1, np.float32),
        np.asarray(w2, np.float32),
        np.asarray(b2, np.float32),
        np.asarray(w3, np.float32),
        np.asarray(b3, np.float32),
    )
    maps = []
    for i in range(8):
        c0 = i * VC3
        maps.append(
            {
                "pose_t": pose_t,
                "basis_t": np.ascontiguousarray(basis_t[:, c0 : c0 + VC3]),
                "mask3": np.ascontiguousarray(mask3[:, c0 : c0 + VC3]),
                "w_all": w_all,
                "bias_all": bias_all,
            }
        )
    return maps


def kernel(pose, basis, mask, w1, b1, w2, b2, w3, b3):
    from concourse.bass_utils import run_bass_kernel_spmd

    if "nc" not in _CACHED:
        _CACHED["nc"] = _build_nc()
    nc = _CACHED["nc"]

    maps = _in_maps(pose, basis, mask, w1, b1, w2, b2, w3, b3)
    r = run_bass_kernel_spmd(nc, maps, core_ids=list(range(8)))
    out = np.concatenate(
        [m["res"].reshape(BATCH, VC, 3) for m in r.results], axis=1
    )
    return np.ascontiguousarray(out[:, :N_VERT, :])



# revision 4
# speedup vs baseline: 1.2039x; 1.2028x over previous
"""BlendShapes model kernel for 8 Trainium2 NeuronCores.

Computation (reference):
    pose_repr = pose[:, 1:].reshape(B, 23, 9) - eye      # (B, J, 9)
    per-joint MLP 9 -> 18 -> 32 -> 8 (ReLU between)      # coff (B, J, 8)
    basis_full = basis[:, None] * mask[:, :, None, None]  # (V, J, 8, 3)
    res = einsum('bjk,vjkc->bvc', coff, basis_full)       # (B, V, 3)

Mapping:
  - Vertices sharded across 8 cores (V=6890 padded to 8*864=6912; VC3=2592
    output columns per core). Each core runs the full MLP with activations
    transposed ([features, batch]) so coff^T feeds the main matmul's
    stationary operand directly.
  - basis*mask (x 2^13, exact) is precomputed on the host as one fp16
    [184, VC3] tensor per core - no on-chip broadcast or multiply.
  - The identity subtraction (pose - eye) is folded into the L1 bias on the
    host: b1_eff = b1 - w1^T eye.
  - L3 matmuls write a stacked [128, 512] / [56, 512] PSUM tile directly
    (tile_position column offsets 32c), so one bias-add per half produces
    coffT_a / coffT_b - no SBUF->SBUF partition-merge DMAs.
  - Main matmul per b-tile is K-grouped: all 6 N-slices with coffT_a
    stationary, then all 6 with coffT_b accumulating (2 stationary switches
    per b-tile). 3 PSUM pair-tiles [128, 1024] rotate with bufs=3; each pair
    is evacuated (ACT/DVE alternating, x 2^-13 descale) and stored as soon
    as its accumulation completes, stores alternating across the two HWDGE
    rings (sync/scalar).
"""

import numpy as np

N_VERT, N_JOINT, BPJ, BATCH = 6890, 23, 8, 1024
VPAD = 6912  # 8 * 864
VC = VPAD // 8  # 864 vertices per core
VC3 = VC * 3  # 2592
K_ALL = N_JOINT * BPJ  # 184
NB = BATCH // 128  # 8 b-tiles

# Unified joint chunking for the MLP: 4 joints per chunk (3 in the tail).
CHUNKS = [(0, 4), (4, 8), (8, 12), (12, 16), (16, 20), (20, 23)]
NCH = len(CHUNKS)


def _offsets(mpj):
    offs, col = [], 0
    for js, je in CHUNKS:
        offs.append(col)
        col += (je - js) * mpj
    return offs, col


W1_OFF, W1_TOT = _offsets(18)  # 414
W2_OFF, W2_TOT = _offsets(32)  # 736
W3_OFF, W3_TOT = _offsets(8)   # 184
W2_OFF = [W1_TOT + o for o in W2_OFF]
W3_OFF = [W1_TOT + W2_TOT + o for o in W3_OFF]
W_COLS = W1_TOT + W2_TOT + W3_TOT  # 1334

# bias_all columns: [0:6] L1 bias (eye term folded in), [6:12] L2 bias,
# [12] L3 bias stacked for chunks 0-3 (128 rows), [13] for chunks 4-5 (56).
BIAS_COLS = 14
BSCALE = 8192.0  # 2**13, exact
DESCALE = 1.0 / 8192.0

# Main matmul N pairs: each pair = one [128, 1024] (2-bank) PSUM tile,
# covering columns [1024p, 1024p + w0 + w1) via two matmuls.
PAIR_W = [(512, 512), (512, 512), (512, 32)]  # covers 2592

_CACHED = {}


def _build_nc():
    import concourse.tile as tile
    from concourse import bacc, mybir
    from contextlib import ExitStack

    dt = mybir.dt
    f32, f16 = dt.float32, dt.float16
    AF = mybir.ActivationFunctionType
    ALU = mybir.AluOpType

    nc = bacc.Bacc(None, target_bir_lowering=False)

    pose_t = nc.dram_tensor("pose_t", [207, BATCH], f16, kind="ExternalInput")
    w_all = nc.dram_tensor("w_all", [128, W_COLS], f16, kind="ExternalInput")
    bias_all = nc.dram_tensor("bias_all", [128, BIAS_COLS], f32, kind="ExternalInput")
    bfm = nc.dram_tensor("bfm", [K_ALL, VC3], f16, kind="ExternalInput")
    res = nc.dram_tensor("res", [BATCH, VC3], f32, kind="ExternalOutput")

    with ExitStack() as ctx:
        tc = ctx.enter_context(tile.TileContext(nc))
        const = ctx.enter_context(tc.tile_pool(name="const", bufs=1))
        work = ctx.enter_context(tc.tile_pool(name="work", bufs=1))
        outp = ctx.enter_context(tc.tile_pool(name="outp", bufs=2))
        pmlp = ctx.enter_context(tc.tile_pool(name="pmlp", bufs=2, space="PSUM"))
        pmain = ctx.enter_context(tc.tile_pool(name="pmain", bufs=3, space="PSUM"))

        # ---- input DMAs. Sync ring: pose chunk 0, weights, bias, remaining
        # pose chunks (critical-path order for the MLP). Scalar ring: the two
        # bfm halves (needed only when the main matmul starts).
        pose_c = []
        for c, (js, je) in enumerate(CHUNKS):
            K = 9 * (je - js)
            t = work.tile([K, BATCH], f16, tag=f"pose_{c}", name=f"pose_{c}")
            pose_c.append(t)
        nc.sync.dma_start(out=pose_c[0][:], in_=pose_t[0:36, :])
        w_sb = const.tile([128, W_COLS], f16, tag="w")
        nc.sync.dma_start(out=w_sb[:], in_=w_all[:, :])
        bias_sb = const.tile([128, BIAS_COLS], f32, tag="bias")
        nc.sync.dma_start(out=bias_sb[:], in_=bias_all[:, :])
        for c, (js, je) in enumerate(CHUNKS):
            if c == 0:
                continue
            K = 9 * (je - js)
            nc.sync.dma_start(out=pose_c[c][:], in_=pose_t[9 * js : 9 * js + K, :])

        bfm_a = work.tile([128, VC3], f16, tag="bfm_a")
        bfm_b = work.tile([56, VC3], f16, tag="bfm_b")
        nc.scalar.dma_start(out=bfm_a[:], in_=bfm[0:128, :])
        nc.scalar.dma_start(out=bfm_b[:], in_=bfm[128:K_ALL, :])

        coffT_a = work.tile([128, BATCH], f16, tag="coffT_a")
        coffT_b = work.tile([56, BATCH], f16, tag="coffT_b")
        h1 = {}
        h2 = {}

        def mlp_half(h):
            hs = slice(h * 512, (h + 1) * 512)
            # L1: 9nj -> 18nj, ReLU(x + b_eff)
            for c, (js, je) in enumerate(CHUNKS):
                nj = je - js
                K, M = 9 * nj, 18 * nj
                off = W1_OFF[c]
                ps = pmlp.tile([M, 512], f32, tag="psmlp", name=f"ps1_{c}_{h}")
                nc.tensor.matmul(
                    ps[:], lhsT=w_sb[0:K, off : off + M], rhs=pose_c[c][:, hs],
                    start=True, stop=True,
                )
                if h == 0:
                    h1[c] = work.tile([M, BATCH], f16, tag=f"h1_{c}", name=f"h1_{c}")
                nc.scalar.activation(
                    h1[c][:, hs], ps[:], AF.Relu, bias=bias_sb[0:M, c : c + 1]
                )
            # L2: 18nj -> 32nj, ReLU; epilogues split ACT/DVE
            for c, (js, je) in enumerate(CHUNKS):
                nj = je - js
                K, M = 18 * nj, 32 * nj
                off = W2_OFF[c]
                ps = pmlp.tile([M, 512], f32, tag="psmlp", name=f"ps2_{c}_{h}")
                nc.tensor.matmul(
                    ps[:], lhsT=w_sb[0:K, off : off + M], rhs=h1[c][:, hs],
                    start=True, stop=True,
                )
                if h == 0:
                    h2[c] = work.tile([M, BATCH], f16, tag=f"h2_{c}", name=f"h2_{c}")
                if c % 2 == 0:
                    nc.scalar.activation(
                        h2[c][:, hs], ps[:], AF.Relu, bias=bias_sb[0:M, 6 + c : 7 + c]
                    )
                else:
                    nc.vector.tensor_scalar(
                        out=h2[c][:, hs], in0=ps[:],
                        scalar1=bias_sb[0:M, 6 + c : 7 + c], scalar2=0.0,
                        op0=ALU.add, op1=ALU.max,
                    )
            # L3: 32nj -> 8nj into stacked PSUM tiles (chunk c at partition
            # 32c), one bias-add each -> coffT_a / coffT_b.
            ps3a = pmlp.tile([128, 512], f32, tag="psmlp", name=f"ps3a_{h}")
            ps3b = pmlp.tile([56, 512], f32, tag="psmlp", name=f"ps3b_{h}")
            for c, (js, je) in enumerate(CHUNKS):
                nj = je - js
                K, M = 32 * nj, 8 * nj
                off = W3_OFF[c]
                if c < 4:
                    r0 = 32 * c
                    dst = ps3a[r0 : r0 + M, :]
                else:
                    r0 = 32 * (c - 4)
                    dst = ps3b[r0 : r0 + M, :]
                nc.tensor.matmul(
                    dst, lhsT=w_sb[0:K, off : off + M], rhs=h2[c][:, hs],
                    start=True, stop=True, tile_position=(0, r0),
                )
            nc.vector.tensor_scalar(
                out=coffT_a[:, hs], in0=ps3a[:], scalar1=bias_sb[0:128, 12:13],
                scalar2=None, op0=ALU.add,
            )
            nc.vector.tensor_scalar(
                out=coffT_b[:, hs], in0=ps3b[:], scalar1=bias_sb[0:56, 13:14],
                scalar2=None, op0=ALU.add,
            )

        def main_btile(bt):
            bsl = slice(bt * 128, (bt + 1) * 128)
            ostrip = outp.tile([128, VC3], f32, tag="ostrip", name=f"ostrip_{bt}")
            ps = [
                pmain.tile([128, 1024], f32, tag="pmain", name=f"ps_{bt}_{p}")
                for p in range(3)
            ]
            # K pass a (coffT_a stationary for all 6 slices), then pass b
            # accumulating; pair p evacuates + stores right after its b MMs.
            for p, (w0, w1) in enumerate(PAIR_W):
                c0 = 1024 * p
                nc.tensor.matmul(
                    ps[p][:, 0:w0], lhsT=coffT_a[:, bsl],
                    rhs=bfm_a[:, c0 : c0 + w0], start=True, stop=False,
                )
                nc.tensor.matmul(
                    ps[p][:, 512 : 512 + w1], lhsT=coffT_a[:, bsl],
                    rhs=bfm_a[:, c0 + 512 : c0 + 512 + w1], start=True, stop=False,
                )
            for p, (w0, w1) in enumerate(PAIR_W):
                c0 = 1024 * p
                nc.tensor.matmul(
                    ps[p][:, 0:w0], lhsT=coffT_b[:, bsl],
                    rhs=bfm_b[:, c0 : c0 + w0], start=False, stop=True,
                )
                nc.tensor.matmul(
                    ps[p][:, 512 : 512 + w1], lhsT=coffT_b[:, bsl],
                    rhs=bfm_b[:, c0 + 512 : c0 + 512 + w1], start=False, stop=True,
                )
                osl = slice(c0, c0 + 512 + w1)
                if (bt + p) % 2 == 0:
                    nc.scalar.activation(
                        ostrip[:, osl], ps[p][:, 0 : 512 + w1], AF.Copy, scale=DESCALE
                    )
                else:
                    nc.vector.tensor_scalar(
                        out=ostrip[:, osl], in0=ps[p][:, 0 : 512 + w1],
                        scalar1=DESCALE, scalar2=None, op0=ALU.mult,
                    )
                ring = nc.sync if (3 * bt + p) % 2 == 0 else nc.scalar
                ring.dma_start(out=res[bsl, osl], in_=ostrip[:, osl])

        mlp_half(0)
        for bt in range(4):
            main_btile(bt)
        mlp_half(1)
        for bt in range(4, NB):
            main_btile(bt)

    nc.finalize()
    return nc


def _pack_host(pose, basis, mask, w1, b1, w2, b2, w3, b3):
    pose_t = np.ascontiguousarray(
        pose[:, 1:].reshape(BATCH, 207).T.astype(np.float16)
    )  # [207, B] rows are (j, i)

    # bfm[8j+k, (v, c)] = mask[v, j] * basis[v, k, c] * 2^13, fp16
    basis_t = basis.transpose(1, 0, 2).reshape(BPJ, N_VERT * 3)  # [k, v3]
    mask3 = np.repeat(mask.T, 3, axis=1)  # [j, v3]
    bfm = np.zeros((K_ALL, VPAD * 3), np.float16)
    bfm[:, : N_VERT * 3] = (
        mask3[:, None, :] * (basis_t * BSCALE)[None, :, :]
    ).reshape(K_ALL, N_VERT * 3).astype(np.float16)

    w_all = np.zeros((128, W_COLS), np.float16)
    for (js, je), o1, o2, o3 in zip(CHUNKS, W1_OFF, W2_OFF, W3_OFF):
        for t, j in enumerate(range(js, je)):
            w_all[t * 9 : (t + 1) * 9, o1 + t * 18 : o1 + (t + 1) * 18] = w1[j]
            w_all[t * 18 : (t + 1) * 18, o2 + t * 32 : o2 + (t + 1) * 32] = w2[j]
            w_all[t * 32 : (t + 1) * 32, o3 + t * 8 : o3 + (t + 1) * 8] = w3[j]

    bias_all = np.zeros((128, BIAS_COLS), np.float32)
    eye = np.zeros(9, np.float32)
    eye[[0, 4, 8]] = 1.0
    b1_eff = b1 - np.einsum("jio,i->jo", w1, eye)  # fold pose - eye into bias
    for c, (js, je) in enumerate(CHUNKS):
        nj = je - js
        bias_all[0 : 18 * nj, c] = b1_eff[js:je].reshape(-1)
        bias_all[0 : 32 * nj, 6 + c] = b2[js:je].reshape(-1)
    bias_all[0:128, 12] = b3[0:16].reshape(-1)
    bias_all[0:56, 13] = b3[16:23].reshape(-1)

    return pose_t, w_all, bias_all, bfm


def _in_maps(pose, basis, mask, w1, b1, w2, b2, w3, b3):
    pose_t, w_all, bias_all, bfm = _pack_host(
        np.asarray(pose, np.float32),
        np.asarray(basis, np.float32),
        np.asarray(mask, np.float32),
        np.asarray(w1, np.float32),
        np.asarray(b1, np.float32),
        np.asarray(w2, np.float32),
        np.asarray(b2, np.float32),
        np.asarray(w3, np.float32),
        np.asarray(b3, np.float32),
    )
    maps = []
    for i in range(8):
        c0 = i * VC3
        maps.append(
            {
                "pose_t": pose_t,
                "w_all": w_all,
                "bias_all": bias_all,
                "bfm": np.ascontiguousarray(bfm[:, c0 : c0 + VC3]),
            }
        )
    return maps


def kernel(pose, basis, mask, w1, b1, w2, b2, w3, b3):
    from concourse.bass_utils import run_bass_kernel_spmd

    if "nc" not in _CACHED:
        _CACHED["nc"] = _build_nc()
    nc = _CACHED["nc"]

    maps = _in_maps(pose, basis, mask, w1, b1, w2, b2, w3, b3)
    r = run_bass_kernel_spmd(nc, maps, core_ids=list(range(8)))
    out = np.concatenate(
        [m["res"].reshape(BATCH, VC, 3) for m in r.results], axis=1
    )
    return np.ascontiguousarray(out[:, :N_VERT, :])
